# revision 24
# baseline (speedup 1.0000x reference)
"""Trainium2 Bass kernel for nn_BasicLSTM: fc0 -> 10x LSTM(768) -> fc1.

Strategy: data-parallel over the 512 windows across 8 cores (64 windows each).
All matmul operands in bf16 (f32 PSUM accumulation, f32 elementwise/state).

Schedule (per core): the tensor engine is the bottleneck (~290k cycles/layer),
so everything else is arranged to keep it streaming (TimelineSim ~1.38 ms,
~93% PE busy):

  - Gate columns are host-permuted into an interleaved layout: 512-col chunk j
    holds [i_j | f_j | g_j | o_j] for h-slice j (128 cols each).  The c/h
    update for slice j starts right after chunk j's matmuls — the elementwise
    tail at each step covers one slice, not the whole gate row.
  - gx (input-side GEMM, batched over (t, b) step pairs with M=128
    stationaries) for layer l+1 is interleaved into layer l's recurrence as
    one burst per m-tile, right after step 2m+1 produced that step pair.  The
    bursts fill the PE idle left by the serial recurrence chain.  One gx
    buffer, overwritten in place after consumption (program order + tile
    dependency tracking make this safe).  w_ih stays resident in a single
    SBUF buffer, reloaded once per layer at t=0 (it is fully consumed by the
    bursts of that layer before the next reload).
  - The "+ gx_t" term is injected into each rec PSUM chunk by a 64x64
    identity matmul (start=True) before the w_hh matmuls accumulate on top.
    This kills the per-step DVE adds and the odd-step partition-shift DMAs of
    the naive layout (odd steps live at partitions 64-127 of gx; a matmul rhs
    can read there, a DVE tensor_tensor against partitions 0-63 cannot).
    (A DVE PSUM-preload variant measured slower: it puts the copy on the
    critical path of every chunk's matmuls.)
  - Rec matmuls are emitted in groups of 2 chunks, k-major within a group.
    h-slice transposes are deferred ("pending") and drained just before the
    first consumer matmul of that k-tile, so PE never waits on the c/h tail.
  - The gx bias add is folded into the PSUM->SBUF copy as a DVE tensor_tensor
    against a bias tile pre-broadcast to all 128 partitions (removes 30 bias
    matmuls per layer).
  - w_hh for layer l+1 and biases prefetch during layer l; fc1 is interleaved
    into layer 9's steps with its two PSUM accumulators held across the layer.
  - Elementwise work is spread across ACT (gates, tanh c, XT writeback), DVE
    (i*g, c add, PSUM->SBUF copies — GpSimd has no PSUM port), and GpSimd
    (f*c, h mult, SBUF-only).
"""
import numpy as np
import ml_dtypes

H = 768
G = 4 * H          # 3072
W = 10             # time steps (window size)
L = 10             # layers
B_FULL = 512
NCORES = 8
BL = B_FULL // NCORES  # 64 windows per core

_CACHE = {}


def build_program(h=H, w=W, nl=L, bl=BL):
    import concourse.mybir as mybir
    import concourse.tile as tile
    from concourse import bacc
    from concourse.masks import make_identity

    F32 = mybir.dt.float32
    BF16 = mybir.dt.bfloat16
    AF = mybir.ActivationFunctionType
    OP = mybir.AluOpType

    g4 = 4 * h
    kt = h // 128           # k-tiles over h (6)
    nch = g4 // 512         # 512-wide chunks over the gate dim (6)
    mt = (w * bl) // 128    # m-tiles over the (t, b) axis (5)
    fh = w * h              # fc1 contraction size
    fn1 = h // 2            # fc1 output chunk (two psum chunks)
    assert h % 128 == 0 and g4 % 512 == 0 and (w * bl) % 128 == 0 and bl == 64
    assert kt == nch  # chunk j's gates act on h-slice j (interleaved layout)

    nc = bacc.Bacc("TRN2", target_bir_lowering=False, debug=False)

    xposT_d = nc.dram_tensor("xposT", [h, w * bl], BF16, kind="ExternalInput")
    fc0wT_d = nc.dram_tensor("fc0wT", [h, h], BF16, kind="ExternalInput")
    fc0b_d = nc.dram_tensor("fc0b", [1, h], BF16, kind="ExternalInput")
    wihT_d = nc.dram_tensor("wihT", [nl, h, g4], BF16, kind="ExternalInput")
    whhT_d = nc.dram_tensor("whhT", [nl, h, g4], BF16, kind="ExternalInput")
    biasT_d = nc.dram_tensor("biasT", [nl, 1, g4], BF16, kind="ExternalInput")
    fc1wT_d = nc.dram_tensor("fc1wT", [fh, h], BF16, kind="ExternalInput")
    fc1bT_d = nc.dram_tensor("fc1bT", [1, h], BF16, kind="ExternalInput")
    out_d = nc.dram_tensor("out", [bl, h], F32, kind="ExternalOutput")

    with tile.TileContext(nc) as tc, \
         tc.tile_pool(name="persist", bufs=1) as pp, \
         tc.tile_pool(name="whhp", bufs=1) as whhp, \
         tc.tile_pool(name="biasp", bufs=1) as biasp, \
         tc.tile_pool(name="gxp", bufs=1) as gxp, \
         tc.tile_pool(name="wstream", bufs=6) as wsp, \
         tc.tile_pool(name="gatep", bufs=4) as gatep, \
         tc.tile_pool(name="tmp", bufs=3) as tp, \
         tc.tile_pool(name="cpool", bufs=2) as cp, \
         tc.tile_pool(name="hpool", bufs=2) as hp, \
         tc.tile_pool(name="outp", bufs=1) as outp, \
         tc.tile_pool(name="psR", bufs=4, space="PSUM") as psR, \
         tc.tile_pool(name="psG", bufs=2, space="PSUM") as psG, \
         tc.tile_pool(name="psT", bufs=2, space="PSUM") as psT:

        # ---- persistent tiles ----
        XT = pp.tile([128, kt, w * bl], BF16)      # h^T / layer-input storage
        onesb = pp.tile([1, 512], BF16)
        nc.vector.memset(onesb[:], 1.0)
        # identity in both partition halves: transposes + gx inject read it at
        # base partition 0 (even steps) or 64 (odd steps — matmul requires
        # lhsT and rhs to share a base partition)
        idb2 = pp.tile([128, 64], BF16)
        make_identity(nc, idb2[0:64, :])
        make_identity(nc, idb2[64:128, :])
        idb = idb2[0:64, :]
        fc0b_sb = pp.tile([1, h], BF16)
        nc.sync.dma_start(fc0b_sb[:], fc0b_d[:])
        fc1b_sb = pp.tile([1, h], BF16)
        nc.sync.dma_start(fc1b_sb[:], fc1bT_d[:])

        whh_bufs = [whhp.tile([128, kt, g4], BF16, name=f"whh{i}") for i in range(2)]
        bias_bufs = [biasp.tile([1, g4], BF16, name=f"bias{i}") for i in range(2)]
        # single-buffered resident w_ih: wih_{l+1} is loaded at layer l t=0 and
        # fully consumed by layer l's gx bursts, before layer l+1 t=0 reloads
        wih_sb = whhp.tile([128, kt, g4], BF16, name="wih_sb")
        # bias broadcast across all 128 partitions (both step parities), so the
        # gx PSUM->SBUF copy folds the bias add (no per-chunk bias matmuls)
        biasf_bufs = [biasp.tile([128, g4], BF16, name=f"biasf{i}") for i in range(2)]

        def emit_bias_broadcast(l):
            """biasf[l%2] <- bias_l broadcast to 128 partitions (via matmul)."""
            for j in range(nch):
                js = slice(j * 512, (j + 1) * 512)
                ps = psG.tile([128, 512], F32, tag="gps", name=f"biasbc_{l}_{j}")
                nc.tensor.matmul(
                    ps[:], onesb[:, 0:128], bias_bufs[l % 2][:, js],
                    start=True, stop=True)
                nc.vector.tensor_copy(biasf_bufs[l % 2][:, js], ps[:])

        # ---- fc0: XT <- fc0_wT.T @ xposT + fc0_b ----
        fc0w = gxp.tile([128, kt, h], BF16, tag="gxA", name="fc0w")
        xpt = gxp.tile([128, kt, w * bl], BF16, tag="gxB", name="xpt")
        for k in range(kt):
            nc.sync.dma_start(
                xpt[:, k, :], xposT_d.rearrange("(k p) c -> p k c", p=128)[:, k, :])
            nc.sync.dma_start(
                fc0w[:, k, :], fc0wT_d.rearrange("(k p) ho -> p k ho", p=128)[:, k, :])
        fc0_chunks = [(c, min(512, w * bl - c)) for c in range(0, w * bl, 512)]
        for m in range(kt):
            for c0, cw in fc0_chunks:
                ps = psG.tile([128, 512], F32, tag="gps", name=f"fc0ps_{m}_{c0}")
                for k in range(kt):
                    nc.tensor.matmul(
                        ps[:, :cw],
                        fc0w[:, k, m * 128:(m + 1) * 128],
                        xpt[:, k, c0:c0 + cw],
                        start=(k == 0), stop=False,
                    )
                nc.tensor.matmul(
                    ps[:, :cw], fc0b_sb[:, m * 128:(m + 1) * 128],
                    onesb[:, 0:cw], start=False, stop=True)
                nc.vector.tensor_copy(XT[:, m, c0:c0 + cw], ps[:, :cw])

        # gx buffer, split in two tiles tag-sharing the SBUF slots of fc0's
        # staging tiles above (fc0's lifetime ends before gx is first written;
        # allocation order matches program order so slot versioning is clean).
        # even steps at partitions 0-63, odd at 64-127.
        gxA = gxp.tile([128, 3, g4], BF16, tag="gxA", name="gxA")
        gxB = gxp.tile([128, mt - 3, g4], BF16, tag="gxB", name="gxB")

        def gx_ap(m):
            return (gxA, m) if m < 3 else (gxB, m - 3)

        pending = []  # deferred (t, j, hh) transposes, drained into the next
                      # consumer's matmul stream just before the k-tile that
                      # needs slice j, so PE never stalls on the c/h tail

        def drain_upto(k):
            while pending and pending[0][1] <= k:
                tt, j, hh_t = pending.pop(0)
                trp = psT.tile([128, 64], BF16, tag="tps", name=f"trp_{tt}_{j}")
                nc.tensor.transpose(trp[:], hh_t[:, j * 128:(j + 1) * 128], idb[:])
                nc.vector.tensor_copy(XT[:, j, tt * 64:(tt + 1) * 64], trp[:])

        def emit_gx(l, mlist):
            """gx[m] <- XT[:, :, m-tile].T @ wih_sb + bias_l, for m in mlist.
            wih_sb must hold wihT_d[l]; biasf[l%2] must hold bias_l broadcast."""
            biasf = biasf_bufs[l % 2]
            for j in range(nch):
                js = slice(j * 512, (j + 1) * 512)
                pss = {m: psG.tile([128, 512], F32, tag="gps", name=f"gxps_{l}_{j}_{m}")
                       for m in mlist}
                for k in range(kt):
                    if j == 0:
                        drain_upto(k)
                    for m in mlist:
                        nc.tensor.matmul(
                            pss[m][:], XT[:, k, m * 128:(m + 1) * 128],
                            wih_sb[:, k, js],
                            start=(k == 0), stop=(k == kt - 1))
                for m in mlist:
                    gxt, ml = gx_ap(m)
                    nc.vector.tensor_tensor(
                        gxt[:, ml, js], pss[m][:], biasf[:, js], OP.add)

        # prefetch layer 0 weights; wih per j-chunk so gx_0's chunk j waits
        # only on its own slice; whh_0 isn't needed until layer 0 step 1
        nc.sync.dma_start(bias_bufs[0][:], biasT_d[0])
        if nl > 1:
            nc.sync.dma_start(bias_bufs[1][:], biasT_d[1])
        for j in range(nch):
            js = slice(j * 512, (j + 1) * 512)
            nc.sync.dma_start(
                wih_sb[:, :, js],
                wihT_d[0].rearrange("(k p) g -> p k g", p=128)[:, :, js])
        nc.sync.dma_start(whh_bufs[0][:], whhT_d[0].rearrange("(k p) g -> p k g", p=128))
        emit_bias_broadcast(0)
        if nl > 1:
            emit_bias_broadcast(1)

        emit_gx(0, list(range(mt)))

        # ---- layers ----
        TANH = AF.Tanh
        SIG = AF.Sigmoid
        groups = [(0, 1), (2, 3), (4, 5)]
        for l in range(nl):
            WHH = whh_bufs[l % 2]
            c_cur = None
            # fc1 accumulators (layer 9 only), held across the whole layer
            if l == nl - 1:
                ps_fc1 = [psG.tile([64, fn1], F32, tag="gps", name=f"fc1ps_{n}")
                          for n in range(2)]
            for t in range(w):
                m, p0 = t // 2, 64 * (t % 2)
                c_new = cp.tile([64, h], F32)
                hh = hp.tile([64, h], BF16)

                def chunk_tail(j, gt, c_prev):
                    # c/h update for h-slice j (gates chunk j); emitted right
                    # after chunk j's gate ACTs so the slice tails pipeline
                    # with later chunks' matmuls/ACTs
                    hs = slice(j * 128, (j + 1) * 128)
                    i_g, f_g = gt[:, 0:128], gt[:, 128:256]
                    g_g, o_g = gt[:, 256:384], gt[:, 384:512]
                    if c_prev is None:
                        nc.vector.tensor_tensor(c_new[:, hs], i_g, g_g, OP.mult)
                    else:
                        t1 = tp.tile([64, 128], F32, tag="t1")
                        nc.vector.tensor_tensor(t1[:], i_g, g_g, OP.mult)
                        t2 = tp.tile([64, 128], F32, tag="t2")
                        nc.gpsimd.tensor_tensor(t2[:], f_g, c_prev[:, hs], OP.mult)
                        nc.vector.tensor_tensor(c_new[:, hs], t1[:], t2[:], OP.add)
                    tc_t = tp.tile([64, 128], F32, tag="tc")
                    nc.scalar.activation(tc_t[:], c_new[:, hs], TANH)
                    nc.gpsimd.tensor_tensor(hh[:, hs], o_g, tc_t[:], OP.mult)
                    pending.append((t, j, hh))

                if t == 0:
                    for j in range(nch):
                        gt = gatep.tile([64, 512], F32, tag="gt")
                        gxt, ml = gx_ap(m)
                        src = gxt[p0:p0 + 64, ml, j * 512:(j + 1) * 512]
                        nc.scalar.activation(gt[:, 0:256], src[:, 0:256], SIG)
                        nc.scalar.activation(gt[:, 256:384], src[:, 256:384], TANH)
                        nc.scalar.activation(gt[:, 384:512], src[:, 384:512], SIG)
                        chunk_tail(j, gt, None)
                else:
                    hT_prev = XT[:, :, (t - 1) * 64:t * 64]
                    lagged = []  # (j, gt) tails emitted one group late so the
                                 # ACT stream isn't stalled on DVE results
                    for gi, grp in enumerate(groups):
                        pss = {j: psR.tile([128, 512], F32, tag="rps", name=f"recps_{l}_{t}_{j}")
                               for j in grp}
                        for j in grp:
                            js = slice(j * 512, (j + 1) * 512)
                            gxt, ml = gx_ap(m)
                            nc.tensor.matmul(
                                pss[j][0:64, :], idb2[p0:p0 + 64, :],
                                gxt[p0:p0 + 64, ml, js],
                                start=True, stop=False)
                        for k in range(kt):
                            if gi == 0:
                                drain_upto(k)
                            for j in grp:
                                js = slice(j * 512, (j + 1) * 512)
                                nc.tensor.matmul(
                                    pss[j][0:64, :],
                                    hT_prev[:, k, :],
                                    WHH[:, k, js],
                                    start=False, stop=(k == kt - 1))
                        for j in grp:
                            gt = gatep.tile([64, 512], F32, tag="gt")
                            ps = pss[j]
                            nc.scalar.activation(gt[:, 0:256], ps[0:64, 0:256], SIG)
                            nc.scalar.activation(gt[:, 256:384], ps[0:64, 256:384], TANH)
                            nc.scalar.activation(gt[:, 384:512], ps[0:64, 384:512], SIG)
                            lagged.append((j, gt))
                        if gi > 0:
                            for (jj, gg) in lagged[:2]:
                                chunk_tail(jj, gg, c_cur)
                            lagged = lagged[2:]
                    for (jj, gg) in lagged:
                        chunk_tail(jj, gg, c_cur)
                # prefetch next layer's weights once per layer, early
                if t == 0 and l + 1 < nl:
                    nc.sync.dma_start(
                        whh_bufs[(l + 1) % 2][:],
                        whhT_d[l + 1].rearrange("(k p) g -> p k g", p=128))
                    for jp in range(3):
                        js = slice(jp * 1024, (jp + 1) * 1024)
                        nc.sync.dma_start(
                            wih_sb[:, :, js],
                            wihT_d[l + 1].rearrange("(k p) g -> p k g", p=128)[:, :, js])
                    if l + 2 < nl:
                        nc.sync.dma_start(bias_bufs[(l + 2) % 2][:], biasT_d[l + 2])
                        emit_bias_broadcast(l + 2)
                c_cur = c_new

                if l + 1 < nl:
                    if t % 2 == 1:
                        emit_gx(l + 1, [t // 2])
                else:
                    # fc1 partial: contract XT[:, s, t-slice] for this t
                    for s in range(kt):
                        drain_upto(s)
                        for n in range(2):
                            ns = slice(n * fn1, (n + 1) * fn1)
                            wk = wsp.tile([128, fn1], BF16, tag="fc1w")
                            nc.sync.dma_start(
                                wk[:], fc1wT_d[(t * kt + s) * 128:(t * kt + s + 1) * 128, ns])
                            nc.tensor.matmul(
                                ps_fc1[n][:], XT[:, s, t * 64:(t + 1) * 64], wk[:],
                                start=(t == 0 and s == 0), stop=False)

        # ---- fc1 finalize ----
        out_sb = outp.tile([64, h], F32)
        for n in range(2):
            ns = slice(n * fn1, (n + 1) * fn1)
            nc.tensor.matmul(
                ps_fc1[n][:], onesb[:, 0:64], fc1b_sb[:, ns], start=False, stop=True)
            nc.vector.tensor_copy(out_sb[:, ns], ps_fc1[n][:])
        nc.sync.dma_start(out_d[:], out_sb[:])

    nc.compile()
    return nc


def _gate_perm(h=H):
    """Column permutation: new chunk j = [i_j | f_j | g_j | o_j] (128 each)."""
    g4 = 4 * h
    kt = h // 128
    perm = np.empty(g4, np.int64)
    for j in range(kt):
        for q in range(4):
            perm[j * 512 + q * 128:j * 512 + (q + 1) * 128] = np.arange(
                q * h + j * 128, q * h + (j + 1) * 128)
    return perm


def prep_inputs_one(inputs, h=H, w=W, nl=L, bl=BL, core=None, x_batch=None):
    """Host-side prep for one shard. inputs keyed as in setup_inputs()."""
    f32 = np.float32
    bf16 = ml_dtypes.bfloat16
    g4 = 4 * h
    if x_batch is None:
        x = np.ascontiguousarray(np.asarray(inputs["x_position"], f32)).reshape(-1, w, h)
        x_batch = x[core * bl:(core + 1) * bl]
    xposT = np.ascontiguousarray(x_batch.transpose(2, 1, 0).reshape(h, w * bl))
    perm = _gate_perm(h)
    wihT = np.asarray(inputs["w_ih"], f32).transpose(0, 2, 1)[:, :, perm]
    whhT = np.asarray(inputs["w_hh"], f32).transpose(0, 2, 1)[:, :, perm]
    biasT = (np.asarray(inputs["b_ih"], f32) + np.asarray(inputs["b_hh"], f32))[:, perm]
    return {
        "xposT": xposT.astype(bf16),
        "fc0wT": np.ascontiguousarray(np.asarray(inputs["fc0_w"], f32).T).astype(bf16),
        "fc0b": np.ascontiguousarray(
            np.asarray(inputs["fc0_b"], f32).reshape(1, h)).astype(bf16),
        "wihT": np.ascontiguousarray(wihT).astype(bf16),
        "whhT": np.ascontiguousarray(whhT).astype(bf16),
        "biasT": np.ascontiguousarray(biasT.reshape(nl, 1, g4)).astype(bf16),
        "fc1wT": np.ascontiguousarray(np.asarray(inputs["fc1_w"], f32).T).astype(bf16),
        "fc1bT": np.ascontiguousarray(
            np.asarray(inputs["fc1_b"], f32).reshape(1, h)).astype(bf16),
    }


def prep_inputs(inputs):
    shared = prep_inputs_one(inputs, core=0)
    x = np.ascontiguousarray(np.asarray(inputs["x_position"], np.float32)).reshape(-1, W, H)
    in_maps = [dict(shared)]
    for c in range(1, NCORES):
        m = dict(shared)
        xc = x[c * BL:(c + 1) * BL]
        m["xposT"] = np.ascontiguousarray(
            xc.transpose(2, 1, 0).reshape(H, W * BL)).astype(ml_dtypes.bfloat16)
        in_maps.append(m)
    return in_maps


def run_on_cores(in_maps, trace=False, **kwargs):
    from concourse.bass_utils import run_bass_kernel_spmd
    if "nc" not in _CACHE:
        _CACHE["nc"] = build_program()
    nc = _CACHE["nc"]
    return run_bass_kernel_spmd(
        nc, in_maps, core_ids=list(range(NCORES)), trace=trace, **kwargs)


def kernel(**inputs) -> np.ndarray:
    in_maps = prep_inputs(inputs)
    res = run_on_cores(in_maps)
    outs = [res.results[c]["out"] for c in range(NCORES)]
    full = np.concatenate(outs, axis=0)            # (512, 768)
    return np.ascontiguousarray(full.reshape(-1, 3).astype(np.float32))


# revision 33
# speedup vs baseline: 1.0712x; 1.0712x over previous
"""Trainium2 Bass kernel for nn_BasicLSTM: fc0 -> 10x LSTM(768) -> fc1.

Strategy: data-parallel over the 512 windows across 8 cores (64 windows each).
All matmul operands in bf16 (f32 PSUM accumulation, f32 elementwise/state).

Schedule (per core): the tensor engine is the bottleneck (~290k cycles/layer),
so everything else is arranged to keep it streaming (TimelineSim ~1.38 ms,
~93% PE busy):

  - Gate columns are host-permuted into an interleaved layout: 512-col chunk j
    holds [i_j | f_j | g_j | o_j] for h-slice j (128 cols each).  The c/h
    update for slice j starts right after chunk j's matmuls — the elementwise
    tail at each step covers one slice, not the whole gate row.
  - gx (input-side GEMM, batched over (t, b) step pairs with M=128
    stationaries) for layer l+1 is interleaved into layer l's recurrence as
    one burst per m-tile, right after step 2m+1 produced that step pair.  The
    bursts fill the PE idle left by the serial recurrence chain.  The m4
    burst is emitted after the NEXT layer's t=0 gate block (its inputs are
    the previous layer's steps 8,9, still intact in XT), so its PE work
    overlaps the t=0 ACT chain.  One gx buffer, overwritten in place after
    consumption (program order + tile dependency tracking make this safe).
    w_ih stays resident in a single SBUF buffer, reloaded once per layer
    right after the m4 burst that consumes the previous contents.
  - The "+ gx_t" term is injected into each rec PSUM chunk by a 64x64
    identity matmul (start=True) before the w_hh matmuls accumulate on top.
    This kills the per-step DVE adds and the odd-step partition-shift DMAs of
    the naive layout (odd steps live at partitions 64-127 of gx; a matmul rhs
    can read there, a DVE tensor_tensor against partitions 0-63 cannot).
    (A DVE PSUM-preload variant measured slower: it puts the copy on the
    critical path of every chunk's matmuls.)
  - Rec matmuls are emitted in groups of 2 chunks, k-major within a group.
    h-slice transposes are deferred ("pending") and drained with one-k-tile
    lookahead (slice k's transpose+copy issue a full matmul ahead of the
    consumer), plus a slice-0 pre-drain at each step end, so PE never waits
    on the c/h tail or on the DVE XT-writeback latency.
  - The gx bias add is folded into the PSUM->SBUF copy as a DVE tensor_tensor
    against a bias tile pre-broadcast to all 128 partitions (removes 30 bias
    matmuls per layer).
  - w_hh for layer l+1 and biases prefetch during layer l; fc1 is interleaved
    into layer 9's steps with its two PSUM accumulators held across the layer.
  - Elementwise work is spread across ACT (gates, tanh c, XT writeback), DVE
    (i*g, c add, PSUM->SBUF copies — GpSimd has no PSUM port), and GpSimd
    (f*c, h mult, SBUF-only).
"""
import numpy as np
import ml_dtypes

H = 768
G = 4 * H          # 3072
W = 10             # time steps (window size)
L = 10             # layers
B_FULL = 512
NCORES = 8
BL = B_FULL // NCORES  # 64 windows per core

_CACHE = {}


def build_program(h=H, w=W, nl=L, bl=BL):
    import concourse.mybir as mybir
    import concourse.tile as tile
    from concourse import bacc
    from concourse.masks import make_identity

    F32 = mybir.dt.float32
    BF16 = mybir.dt.bfloat16
    AF = mybir.ActivationFunctionType
    OP = mybir.AluOpType

    g4 = 4 * h
    kt = h // 128           # k-tiles over h (6)
    nch = g4 // 512         # 512-wide chunks over the gate dim (6)
    mt = (w * bl) // 128    # m-tiles over the (t, b) axis (5)
    fh = w * h              # fc1 contraction size
    fn1 = h // 2            # fc1 output chunk (two psum chunks)
    assert h % 128 == 0 and g4 % 512 == 0 and (w * bl) % 128 == 0 and bl == 64
    assert kt == nch  # chunk j's gates act on h-slice j (interleaved layout)

    nc = bacc.Bacc("TRN2", target_bir_lowering=False, debug=False)

    xposT_d = nc.dram_tensor("xposT", [h, w * bl], BF16, kind="ExternalInput")
    fc0wT_d = nc.dram_tensor("fc0wT", [h, h], BF16, kind="ExternalInput")
    fc0b_d = nc.dram_tensor("fc0b", [1, h], BF16, kind="ExternalInput")
    wihT_d = nc.dram_tensor("wihT", [nl, h, g4], BF16, kind="ExternalInput")
    whhT_d = nc.dram_tensor("whhT", [nl, h, g4], BF16, kind="ExternalInput")
    biasT_d = nc.dram_tensor("biasT", [nl, 1, g4], BF16, kind="ExternalInput")
    fc1wT_d = nc.dram_tensor("fc1wT", [fh, h], BF16, kind="ExternalInput")
    fc1bT_d = nc.dram_tensor("fc1bT", [1, h], BF16, kind="ExternalInput")
    out_d = nc.dram_tensor("out", [bl, h], F32, kind="ExternalOutput")

    with tile.TileContext(nc) as tc, \
         tc.tile_pool(name="persist", bufs=1) as pp, \
         tc.tile_pool(name="whhp", bufs=1) as whhp, \
         tc.tile_pool(name="biasp", bufs=1) as biasp, \
         tc.tile_pool(name="gxp", bufs=1) as gxp, \
         tc.tile_pool(name="wstream", bufs=6) as wsp, \
         tc.tile_pool(name="gatep", bufs=4) as gatep, \
         tc.tile_pool(name="tmp", bufs=3) as tp, \
         tc.tile_pool(name="cpool", bufs=2) as cp, \
         tc.tile_pool(name="hpool", bufs=2) as hp, \
         tc.tile_pool(name="outp", bufs=1) as outp, \
         tc.tile_pool(name="psR", bufs=4, space="PSUM") as psR, \
         tc.tile_pool(name="psG", bufs=2, space="PSUM") as psG, \
         tc.tile_pool(name="psT", bufs=2, space="PSUM") as psT:

        # ---- persistent tiles ----
        XT = pp.tile([128, kt, w * bl], BF16)      # h^T / layer-input storage
        onesb = pp.tile([1, 512], BF16)
        nc.vector.memset(onesb[:], 1.0)
        # identity in both partition halves: transposes + gx inject read it at
        # base partition 0 (even steps) or 64 (odd steps — matmul requires
        # lhsT and rhs to share a base partition)
        idb2 = pp.tile([128, 64], BF16)
        make_identity(nc, idb2[0:64, :])
        make_identity(nc, idb2[64:128, :])
        idb = idb2[0:64, :]
        fc0b_sb = pp.tile([1, h], BF16)
        nc.sync.dma_start(fc0b_sb[:], fc0b_d[:])
        fc1b_sb = pp.tile([1, h], BF16)
        nc.sync.dma_start(fc1b_sb[:], fc1bT_d[:])

        whh_bufs = [whhp.tile([128, kt, g4], BF16, name=f"whh{i}") for i in range(2)]
        bias_bufs = [biasp.tile([1, g4], BF16, name=f"bias{i}") for i in range(2)]
        # single-buffered resident w_ih: wih_{l+1} is loaded at layer l t=0 and
        # fully consumed by layer l's gx bursts, before layer l+1 t=0 reloads
        wih_sb = whhp.tile([128, kt, g4], BF16, name="wih_sb")
        # bias broadcast across all 128 partitions (both step parities), so the
        # gx PSUM->SBUF copy folds the bias add (no per-chunk bias matmuls)
        biasf_bufs = [biasp.tile([128, g4], BF16, name=f"biasf{i}") for i in range(2)]

        def emit_bias_broadcast(l):
            """biasf[l%2] <- bias_l broadcast to 128 partitions (via matmul)."""
            for j in range(nch):
                js = slice(j * 512, (j + 1) * 512)
                ps = psG.tile([128, 512], F32, tag="gps", name=f"biasbc_{l}_{j}")
                nc.tensor.matmul(
                    ps[:], onesb[:, 0:128], bias_bufs[l % 2][:, js],
                    start=True, stop=True)
                nc.vector.tensor_copy(biasf_bufs[l % 2][:, js], ps[:])

        # ---- fc0: XT <- fc0_wT.T @ xposT + fc0_b ----
        fc0w = gxp.tile([128, kt, h], BF16, tag="gxA", name="fc0w")
        xpt = gxp.tile([128, kt, w * bl], BF16, tag="gxB", name="xpt")
        for k in range(kt):
            nc.sync.dma_start(
                xpt[:, k, :], xposT_d.rearrange("(k p) c -> p k c", p=128)[:, k, :])
            nc.sync.dma_start(
                fc0w[:, k, :], fc0wT_d.rearrange("(k p) ho -> p k ho", p=128)[:, k, :])
        fc0_chunks = [(c, min(512, w * bl - c)) for c in range(0, w * bl, 512)]
        for m in range(kt):
            for c0, cw in fc0_chunks:
                ps = psG.tile([128, 512], F32, tag="gps", name=f"fc0ps_{m}_{c0}")
                for k in range(kt):
                    nc.tensor.matmul(
                        ps[:, :cw],
                        fc0w[:, k, m * 128:(m + 1) * 128],
                        xpt[:, k, c0:c0 + cw],
                        start=(k == 0), stop=False,
                    )
                nc.tensor.matmul(
                    ps[:, :cw], fc0b_sb[:, m * 128:(m + 1) * 128],
                    onesb[:, 0:cw], start=False, stop=True)
                nc.vector.tensor_copy(XT[:, m, c0:c0 + cw], ps[:, :cw])

        # gx buffer, split in two tiles tag-sharing the SBUF slots of fc0's
        # staging tiles above (fc0's lifetime ends before gx is first written;
        # allocation order matches program order so slot versioning is clean).
        # even steps at partitions 0-63, odd at 64-127.
        gxA = gxp.tile([128, 3, g4], BF16, tag="gxA", name="gxA")
        gxB = gxp.tile([128, mt - 3, g4], BF16, tag="gxB", name="gxB")

        def gx_ap(m):
            return (gxA, m) if m < 3 else (gxB, m - 3)

        pending = []  # deferred (t, j, hh) transposes, drained into the next
                      # consumer's matmul stream just before the k-tile that
                      # needs slice j, so PE never stalls on the c/h tail

        def drain_upto(k):
            while pending and pending[0][1] <= k:
                tt, j, hh_t = pending.pop(0)
                trp = psT.tile([128, 64], BF16, tag="tps", name=f"trp_{tt}_{j}")
                nc.tensor.transpose(trp[:], hh_t[:, j * 128:(j + 1) * 128], idb[:])
                nc.vector.tensor_copy(XT[:, j, tt * 64:(tt + 1) * 64], trp[:])

        def emit_gx(l, mlist):
            """gx[m] <- XT[:, :, m-tile].T @ wih_sb + bias_l, for m in mlist.
            wih_sb must hold wihT_d[l]; biasf[l%2] must hold bias_l broadcast."""
            biasf = biasf_bufs[l % 2]
            drain_upto(0)
            for j in range(nch):
                js = slice(j * 512, (j + 1) * 512)
                pss = {m: psG.tile([128, 512], F32, tag="gps", name=f"gxps_{l}_{j}_{m}")
                       for m in mlist}
                for k in range(kt):
                    if j == 0:
                        # one-k lookahead: slice k was already drained at the
                        # previous iteration, so its XT copy has had a full
                        # matmul's time to land before this MM consumes it
                        drain_upto(k + 1)
                    for m in mlist:
                        nc.tensor.matmul(
                            pss[m][:], XT[:, k, m * 128:(m + 1) * 128],
                            wih_sb[:, k, js],
                            start=(k == 0), stop=(k == kt - 1))
                for m in mlist:
                    gxt, ml = gx_ap(m)
                    nc.vector.tensor_tensor(
                        gxt[:, ml, js], pss[m][:], biasf[:, js], OP.add)

        # prefetch layer 0 weights; wih per j-chunk so gx_0's chunk j waits
        # only on its own slice; whh_0 isn't needed until layer 0 step 1
        nc.sync.dma_start(bias_bufs[0][:], biasT_d[0])
        if nl > 1:
            nc.sync.dma_start(bias_bufs[1][:], biasT_d[1])
        for j in range(nch):
            js = slice(j * 512, (j + 1) * 512)
            nc.sync.dma_start(
                wih_sb[:, :, js],
                wihT_d[0].rearrange("(k p) g -> p k g", p=128)[:, :, js])
        nc.sync.dma_start(whh_bufs[0][:], whhT_d[0].rearrange("(k p) g -> p k g", p=128))
        emit_bias_broadcast(0)
        if nl > 1:
            emit_bias_broadcast(1)

        emit_gx(0, list(range(mt)))

        # ---- layers ----
        TANH = AF.Tanh
        SIG = AF.Sigmoid
        groups = [(0, 1), (2, 3), (4, 5)]
        for l in range(nl):
            WHH = whh_bufs[l % 2]
            c_cur = None
            # fc1 accumulators (layer 9 only), held across the whole layer
            if l == nl - 1:
                ps_fc1 = [psG.tile([64, fn1], F32, tag="gps", name=f"fc1ps_{n}")
                          for n in range(2)]
            for t in range(w):
                m, p0 = t // 2, 64 * (t % 2)
                c_new = cp.tile([64, h], F32)
                hh = hp.tile([64, h], BF16)

                def chunk_tail(j, gt, c_prev):
                    # c/h update for h-slice j (gates chunk j); emitted right
                    # after chunk j's gate ACTs so the slice tails pipeline
                    # with later chunks' matmuls/ACTs
                    hs = slice(j * 128, (j + 1) * 128)
                    i_g, f_g = gt[:, 0:128], gt[:, 128:256]
                    g_g, o_g = gt[:, 256:384], gt[:, 384:512]
                    if c_prev is None:
                        nc.vector.tensor_tensor(c_new[:, hs], i_g, g_g, OP.mult)
                    else:
                        t1 = tp.tile([64, 128], F32, tag="t1")
                        nc.vector.tensor_tensor(t1[:], i_g, g_g, OP.mult)
                        t2 = tp.tile([64, 128], F32, tag="t2")
                        nc.gpsimd.tensor_tensor(t2[:], f_g, c_prev[:, hs], OP.mult)
                        nc.vector.tensor_tensor(c_new[:, hs], t1[:], t2[:], OP.add)
                    tc_t = tp.tile([64, 128], F32, tag="tc")
                    nc.scalar.activation(tc_t[:], c_new[:, hs], TANH)
                    nc.gpsimd.tensor_tensor(hh[:, hs], o_g, tc_t[:], OP.mult)
                    pending.append((t, j, hh))

                if t == 0:
                    for j in range(nch):
                        gt = gatep.tile([64, 512], F32, tag="gt")
                        gxt, ml = gx_ap(m)
                        src = gxt[p0:p0 + 64, ml, j * 512:(j + 1) * 512]
                        nc.scalar.activation(gt[:, 0:256], src[:, 0:256], SIG)
                        nc.scalar.activation(gt[:, 256:384], src[:, 256:384], TANH)
                        nc.scalar.activation(gt[:, 384:512], src[:, 384:512], SIG)
                        chunk_tail(j, gt, None)
                else:
                    hT_prev = XT[:, :, (t - 1) * 64:t * 64]
                    lagged = []  # (j, gt) tails emitted one group late so the
                                 # ACT stream isn't stalled on DVE results
                    for gi, grp in enumerate(groups):
                        pss = {j: psR.tile([128, 512], F32, tag="rps", name=f"recps_{l}_{t}_{j}")
                               for j in grp}
                        for j in grp:
                            js = slice(j * 512, (j + 1) * 512)
                            gxt, ml = gx_ap(m)
                            nc.tensor.matmul(
                                pss[j][0:64, :], idb2[p0:p0 + 64, :],
                                gxt[p0:p0 + 64, ml, js],
                                start=True, stop=False)
                        for k in range(kt):
                            if gi == 0:
                                drain_upto(k + 1)
                            for j in grp:
                                js = slice(j * 512, (j + 1) * 512)
                                nc.tensor.matmul(
                                    pss[j][0:64, :],
                                    hT_prev[:, k, :],
                                    WHH[:, k, js],
                                    start=False, stop=(k == kt - 1))
                        for j in grp:
                            gt = gatep.tile([64, 512], F32, tag="gt")
                            ps = pss[j]
                            nc.scalar.activation(gt[:, 0:256], ps[0:64, 0:256], SIG)
                            nc.scalar.activation(gt[:, 256:384], ps[0:64, 256:384], TANH)
                            nc.scalar.activation(gt[:, 384:512], ps[0:64, 384:512], SIG)
                            lagged.append((j, gt))
                        if gi > 0:
                            for (jj, gg) in lagged[:2]:
                                chunk_tail(jj, gg, c_cur)
                            lagged = lagged[2:]
                    for (jj, gg) in lagged:
                        chunk_tail(jj, gg, c_cur)
                # this layer's last gx m-tile (m4, from the previous layer's
                # steps 8,9) — emitted after t=0's gate/tail block so the PE
                # work overlaps the t=0 ACT chain instead of preceding it
                if t == 0 and l > 0:
                    emit_gx(l, [4])
                # prefetch next layer's weights once per layer, early
                if t == 0 and l + 1 < nl:
                    nc.sync.dma_start(
                        whh_bufs[(l + 1) % 2][:],
                        whhT_d[l + 1].rearrange("(k p) g -> p k g", p=128))
                    for jp in range(3):
                        js = slice(jp * 1024, (jp + 1) * 1024)
                        nc.sync.dma_start(
                            wih_sb[:, :, js],
                            wihT_d[l + 1].rearrange("(k p) g -> p k g", p=128)[:, :, js])
                    if l + 2 < nl:
                        nc.sync.dma_start(bias_bufs[(l + 2) % 2][:], biasT_d[l + 2])
                        emit_bias_broadcast(l + 2)
                c_cur = c_new
                drain_upto(0)

                if l + 1 < nl:
                    if t % 2 == 1 and t < w - 1:
                        emit_gx(l + 1, [t // 2])
                else:
                    # fc1 partial: contract XT[:, s, t-slice] for this t
                    for s in range(kt):
                        drain_upto(s + 1)
                        for n in range(2):
                            ns = slice(n * fn1, (n + 1) * fn1)
                            wk = wsp.tile([128, fn1], BF16, tag="fc1w")
                            nc.sync.dma_start(
                                wk[:], fc1wT_d[(t * kt + s) * 128:(t * kt + s + 1) * 128, ns])
                            nc.tensor.matmul(
                                ps_fc1[n][:], XT[:, s, t * 64:(t + 1) * 64], wk[:],
                                start=(t == 0 and s == 0), stop=False)

        # ---- fc1 finalize ----
        out_sb = outp.tile([64, h], F32)
        for n in range(2):
            ns = slice(n * fn1, (n + 1) * fn1)
            nc.tensor.matmul(
                ps_fc1[n][:], onesb[:, 0:64], fc1b_sb[:, ns], start=False, stop=True)
            nc.vector.tensor_copy(out_sb[:, ns], ps_fc1[n][:])
        nc.sync.dma_start(out_d[:], out_sb[:])

    nc.compile()
    return nc


def _gate_perm(h=H):
    """Column permutation: new chunk j = [i_j | f_j | g_j | o_j] (128 each)."""
    g4 = 4 * h
    kt = h // 128
    perm = np.empty(g4, np.int64)
    for j in range(kt):
        for q in range(4):
            perm[j * 512 + q * 128:j * 512 + (q + 1) * 128] = np.arange(
                q * h + j * 128, q * h + (j + 1) * 128)
    return perm


def prep_inputs_one(inputs, h=H, w=W, nl=L, bl=BL, core=None, x_batch=None):
    """Host-side prep for one shard. inputs keyed as in setup_inputs()."""
    f32 = np.float32
    bf16 = ml_dtypes.bfloat16
    g4 = 4 * h
    if x_batch is None:
        x = np.ascontiguousarray(np.asarray(inputs["x_position"], f32)).reshape(-1, w, h)
        x_batch = x[core * bl:(core + 1) * bl]
    xposT = np.ascontiguousarray(x_batch.transpose(2, 1, 0).reshape(h, w * bl))
    perm = _gate_perm(h)
    wihT = np.asarray(inputs["w_ih"], f32).transpose(0, 2, 1)[:, :, perm]
    whhT = np.asarray(inputs["w_hh"], f32).transpose(0, 2, 1)[:, :, perm]
    biasT = (np.asarray(inputs["b_ih"], f32) + np.asarray(inputs["b_hh"], f32))[:, perm]
    return {
        "xposT": xposT.astype(bf16),
        "fc0wT": np.ascontiguousarray(np.asarray(inputs["fc0_w"], f32).T).astype(bf16),
        "fc0b": np.ascontiguousarray(
            np.asarray(inputs["fc0_b"], f32).reshape(1, h)).astype(bf16),
        "wihT": np.ascontiguousarray(wihT).astype(bf16),
        "whhT": np.ascontiguousarray(whhT).astype(bf16),
        "biasT": np.ascontiguousarray(biasT.reshape(nl, 1, g4)).astype(bf16),
        "fc1wT": np.ascontiguousarray(np.asarray(inputs["fc1_w"], f32).T).astype(bf16),
        "fc1bT": np.ascontiguousarray(
            np.asarray(inputs["fc1_b"], f32).reshape(1, h)).astype(bf16),
    }


def prep_inputs(inputs):
    shared = prep_inputs_one(inputs, core=0)
    x = np.ascontiguousarray(np.asarray(inputs["x_position"], np.float32)).reshape(-1, W, H)
    in_maps = [dict(shared)]
    for c in range(1, NCORES):
        m = dict(shared)
        xc = x[c * BL:(c + 1) * BL]
        m["xposT"] = np.ascontiguousarray(
            xc.transpose(2, 1, 0).reshape(H, W * BL)).astype(ml_dtypes.bfloat16)
        in_maps.append(m)
    return in_maps


def run_on_cores(in_maps, trace=False, **kwargs):
    from concourse.bass_utils import run_bass_kernel_spmd
    if "nc" not in _CACHE:
        _CACHE["nc"] = build_program()
    nc = _CACHE["nc"]
    return run_bass_kernel_spmd(
        nc, in_maps, core_ids=list(range(NCORES)), trace=trace, **kwargs)


def kernel(**inputs) -> np.ndarray:
    in_maps = prep_inputs(inputs)
    res = run_on_cores(in_maps)
    outs = [res.results[c]["out"] for c in range(NCORES)]
    full = np.concatenate(outs, axis=0)            # (512, 768)
    return np.ascontiguousarray(full.reshape(-1, 3).astype(np.float32))


# revision 34
# speedup vs baseline: 1.2160x; 1.1352x over previous
"""Trainium2 Bass kernel for nn_BasicLSTM: fc0 -> 10x LSTM(768) -> fc1.

Strategy: data-parallel over the 512 windows across 8 cores (64 windows each).
All matmul operands in bf16 (f32 PSUM accumulation, f32 elementwise/state).

Schedule (per core): the tensor engine is the bottleneck (~290k cycles/layer),
so everything else is arranged to keep it streaming (TimelineSim ~1.38 ms,
~93% PE busy):

  - Gate columns are host-permuted into an interleaved layout: 512-col chunk j
    holds [i_j | f_j | g_j | o_j] for h-slice j (128 cols each).  The c/h
    update for slice j starts right after chunk j's matmuls — the elementwise
    tail at each step covers one slice, not the whole gate row.
  - gx (input-side GEMM, batched over (t, b) step pairs with M=128
    stationaries) for layer l+1 is interleaved into layer l's recurrence as
    one burst per m-tile, right after step 2m+1 produced that step pair.  The
    bursts fill the PE idle left by the serial recurrence chain.  The m4
    burst is emitted after the NEXT layer's t=0 gate block (its inputs are
    the previous layer's steps 8,9, still intact in XT), so its PE work
    overlaps the t=0 ACT chain.  One gx buffer, overwritten in place after
    consumption (program order + tile dependency tracking make this safe).
    w_ih stays resident in a single SBUF buffer, reloaded once per layer
    right after the m4 burst that consumes the previous contents.
  - The "+ gx_t" term is injected into each rec PSUM chunk by a 64x64
    identity matmul (start=True) before the w_hh matmuls accumulate on top.
    This kills the per-step DVE adds and the odd-step partition-shift DMAs of
    the naive layout (odd steps live at partitions 64-127 of gx; a matmul rhs
    can read there, a DVE tensor_tensor against partitions 0-63 cannot).
    (A DVE PSUM-preload variant measured slower: it puts the copy on the
    critical path of every chunk's matmuls.)
  - Rec matmuls are emitted in groups of 2 chunks, k-major within a group.
    h-slice transposes are deferred ("pending") and drained with one-k-tile
    lookahead (slice k's transpose+copy issue a full matmul ahead of the
    consumer), plus a slice-0 pre-drain at each step end, so PE never waits
    on the c/h tail or on the DVE XT-writeback latency.
  - The gx bias add is folded into the PSUM->SBUF copy as a DVE tensor_tensor
    against a bias tile pre-broadcast to all 128 partitions (removes 30 bias
    matmuls per layer).
  - w_hh for layer l+1 and biases prefetch during layer l; fc1 is interleaved
    into layer 9's steps with its two PSUM accumulators held across the layer.
  - Elementwise work is spread across ACT (gates, tanh c, XT writeback), DVE
    (i*g, c add, PSUM->SBUF copies — GpSimd has no PSUM port), and GpSimd
    (f*c, h mult, SBUF-only).
"""
import numpy as np
import ml_dtypes

H = 768
G = 4 * H          # 3072
W = 10             # time steps (window size)
L = 10             # layers
B_FULL = 512
NCORES = 8
BL = B_FULL // NCORES  # 64 windows per core

_CACHE = {}


def build_program(h=H, w=W, nl=L, bl=BL):
    import concourse.mybir as mybir
    import concourse.tile as tile
    from concourse import bacc
    from concourse.masks import make_identity

    F32 = mybir.dt.float32
    BF16 = mybir.dt.bfloat16
    FP8 = mybir.dt.float8e4
    DR = mybir.MatmulPerfMode.DoubleRow
    WSCALE = 32.0
    AF = mybir.ActivationFunctionType
    OP = mybir.AluOpType

    g4 = 4 * h
    kt = h // 128           # k-tiles over h (6)
    nch = g4 // 512         # 512-wide chunks over the gate dim (6)
    mt = (w * bl) // 128    # m-tiles over the (t, b) axis (5)
    fh = w * h              # fc1 contraction size
    fn1 = h // 2            # fc1 output chunk (two psum chunks)
    assert h % 128 == 0 and g4 % 512 == 0 and (w * bl) % 128 == 0 and bl == 64
    assert kt == nch  # chunk j's gates act on h-slice j (interleaved layout)

    nc = bacc.Bacc("TRN2", target_bir_lowering=False, debug=False)

    xposT_d = nc.dram_tensor("xposT", [h, w * bl], BF16, kind="ExternalInput")
    fc0wT_d = nc.dram_tensor("fc0wT", [h, h], BF16, kind="ExternalInput")
    fc0b_d = nc.dram_tensor("fc0b", [1, h], BF16, kind="ExternalInput")
    # fp8 weights in DoubleRow block layout: row p, block i (of 3), plane q
    # (of 2) holds w[h = 256*i + 128*q + p, gate], pre-scaled by 32 on the host
    # (sigma=0.02 weights sit in fp8e4m3's subnormal range unscaled)
    wihT8_d = nc.dram_tensor("wihT8", [nl, 128, 6 * g4], FP8, kind="ExternalInput")
    whhT8_d = nc.dram_tensor("whhT8", [nl, 128, 6 * g4], FP8, kind="ExternalInput")
    biasT_d = nc.dram_tensor("biasT", [nl, 1, g4], BF16, kind="ExternalInput")
    fc1wT_d = nc.dram_tensor("fc1wT", [fh, h], BF16, kind="ExternalInput")
    fc1bT_d = nc.dram_tensor("fc1bT", [1, h], BF16, kind="ExternalInput")
    out_d = nc.dram_tensor("out", [bl, h], F32, kind="ExternalOutput")

    with tile.TileContext(nc) as tc, \
         tc.tile_pool(name="persist", bufs=1) as pp, \
         tc.tile_pool(name="whhp", bufs=1) as whhp, \
         tc.tile_pool(name="biasp", bufs=1) as biasp, \
         tc.tile_pool(name="gxp", bufs=1) as gxp, \
         tc.tile_pool(name="wstream", bufs=6) as wsp, \
         tc.tile_pool(name="gatep", bufs=4) as gatep, \
         tc.tile_pool(name="tmp", bufs=3) as tp, \
         tc.tile_pool(name="cpool", bufs=2) as cp, \
         tc.tile_pool(name="hpool", bufs=2) as hp, \
         tc.tile_pool(name="outp", bufs=1) as outp, \
         tc.tile_pool(name="psR", bufs=4, space="PSUM") as psR, \
         tc.tile_pool(name="psG", bufs=2, space="PSUM") as psG, \
         tc.tile_pool(name="psT", bufs=2, space="PSUM") as psT:

        # ---- persistent tiles ----
        XT = pp.tile([128, kt, w * bl], BF16)      # h^T / layer-input storage
        onesb = pp.tile([1, 512], BF16)
        nc.vector.memset(onesb[:], 1.0)
        # identity in both partition halves: transposes + gx inject read it at
        # base partition 0 (even steps) or 64 (odd steps — matmul requires
        # lhsT and rhs to share a base partition)
        idb2 = pp.tile([128, 64], BF16)
        make_identity(nc, idb2[0:64, :])
        make_identity(nc, idb2[64:128, :])
        idb = idb2[0:64, :]
        fc0b_sb = pp.tile([1, h], BF16)
        nc.sync.dma_start(fc0b_sb[:], fc0b_d[:])
        fc1b_sb = pp.tile([1, h], BF16)
        nc.sync.dma_start(fc1b_sb[:], fc1bT_d[:])

        whh_bufs = [whhp.tile([128, kt, g4], FP8, name=f"whh{i}") for i in range(2)]
        bias_bufs = [biasp.tile([1, g4], BF16, name=f"bias{i}") for i in range(2)]
        # single-buffered resident w_ih: wih_{l+1} is loaded at layer l t=0 and
        # fully consumed by layer l's gx bursts, before layer l+1 t=0 reloads
        wih_sb = whhp.tile([128, kt, g4], FP8, name="wih_sb")
        # fp8 twin of XT (same slice indexing) — the DoubleRow lhsT for both
        # the rec and gx matmuls; plane pair for block i = slices 2i, 2i+1
        XT8 = pp.tile([128, kt, w * bl], FP8, name="XT8")
        # identity scaled by WSCALE for the gx inject, so the injected gx
        # matches the x32 weight scale (the gate ACTs then descale by 1/32);
        # the plain identity idb stays for transposes (CoreSim requires a
        # 0/1 permutation matrix there)
        idb32 = pp.tile([128, 64], BF16, name="idb32")
        make_identity(nc, idb32[0:64, :])
        make_identity(nc, idb32[64:128, :])
        nc.gpsimd.tensor_scalar_mul(idb32[:], idb32[:], WSCALE)
        # bias broadcast across all 128 partitions (both step parities), so the
        # gx PSUM->SBUF copy folds the bias add (no per-chunk bias matmuls)
        biasf_bufs = [biasp.tile([128, g4], BF16, name=f"biasf{i}") for i in range(2)]

        def emit_bias_broadcast(l):
            """biasf[l%2] <- bias_l broadcast to 128 partitions (via matmul)."""
            for j in range(nch):
                js = slice(j * 512, (j + 1) * 512)
                ps = psG.tile([128, 512], F32, tag="gps", name=f"biasbc_{l}_{j}")
                nc.tensor.matmul(
                    ps[:], onesb[:, 0:128], bias_bufs[l % 2][:, js],
                    start=True, stop=True)
                nc.vector.tensor_copy(biasf_bufs[l % 2][:, js], ps[:])

        # ---- fc0: XT <- fc0_wT.T @ xposT + fc0_b ----
        fc0w = gxp.tile([128, kt, h], BF16, tag="gxA", name="fc0w")
        xpt = gxp.tile([128, kt, w * bl], BF16, tag="gxB", name="xpt")
        for k in range(kt):
            nc.sync.dma_start(
                xpt[:, k, :], xposT_d.rearrange("(k p) c -> p k c", p=128)[:, k, :])
            nc.sync.dma_start(
                fc0w[:, k, :], fc0wT_d.rearrange("(k p) ho -> p k ho", p=128)[:, k, :])
        fc0_chunks = [(c, min(512, w * bl - c)) for c in range(0, w * bl, 512)]
        for m in range(kt):
            for c0, cw in fc0_chunks:
                ps = psG.tile([128, 512], F32, tag="gps", name=f"fc0ps_{m}_{c0}")
                for k in range(kt):
                    nc.tensor.matmul(
                        ps[:, :cw],
                        fc0w[:, k, m * 128:(m + 1) * 128],
                        xpt[:, k, c0:c0 + cw],
                        start=(k == 0), stop=False,
                    )
                nc.tensor.matmul(
                    ps[:, :cw], fc0b_sb[:, m * 128:(m + 1) * 128],
                    onesb[:, 0:cw], start=False, stop=True)
                nc.vector.tensor_copy(XT[:, m, c0:c0 + cw], ps[:, :cw])
                nc.scalar.copy(XT8[:, m, c0:c0 + cw], ps[:, :cw])

        # gx buffer, split in two tiles tag-sharing the SBUF slots of fc0's
        # staging tiles above (fc0's lifetime ends before gx is first written;
        # allocation order matches program order so slot versioning is clean).
        # even steps at partitions 0-63, odd at 64-127.
        gxA = gxp.tile([128, 3, g4], BF16, tag="gxA", name="gxA")
        gxB = gxp.tile([128, mt - 3, g4], BF16, tag="gxB", name="gxB")

        def gx_ap(m):
            return (gxA, m) if m < 3 else (gxB, m - 3)

        pending = []  # deferred (t, j, hh) transposes, drained into the next
                      # consumer's matmul stream just before the k-tile that
                      # needs slice j, so PE never stalls on the c/h tail

        def drain_upto(k):
            while pending and pending[0][1] <= k:
                tt, j, hh_t = pending.pop(0)
                trp = psT.tile([128, 64], BF16, tag="tps", name=f"trp_{tt}_{j}")
                nc.tensor.transpose(trp[:], hh_t[:, j * 128:(j + 1) * 128], idb[:])
                nc.vector.tensor_copy(XT[:, j, tt * 64:(tt + 1) * 64], trp[:])
                nc.scalar.copy(XT8[:, j, tt * 64:(tt + 1) * 64], trp[:])

        def emit_gx(l, mlist):
            """gx[m] <- XT[:, :, m-tile].T @ wih_sb + bias_l, for m in mlist.
            wih_sb must hold wihT_d[l]; biasf[l%2] must hold bias_l broadcast."""
            biasf = biasf_bufs[l % 2]
            drain_upto(0)
            for j in range(nch):
                js = slice(j * 512, (j + 1) * 512)
                pss = {m: psG.tile([128, 512], F32, tag="gps", name=f"gxps_{l}_{j}_{m}")
                       for m in mlist}
                for i in range(kt // 2):
                    if j == 0:
                        # lookahead drain: block i consumes slices 2i, 2i+1
                        drain_upto(min(2 * i + 3, kt - 1))
                    for m in mlist:
                        nc.tensor.matmul(
                            pss[m][:],
                            XT8[:, 2 * i:2 * i + 2, m * 128:(m + 1) * 128],
                            wih_sb[:, 2 * i:2 * i + 2, js],
                            start=(i == 0), stop=(i == kt // 2 - 1),
                            perf_mode=DR)
                for m in mlist:
                    gxt, ml = gx_ap(m)
                    nc.vector.scalar_tensor_tensor(
                        gxt[:, ml, js], pss[m][:], 1.0 / WSCALE, biasf[:, js],
                        OP.mult, OP.add)

        # prefetch layer 0 weights; wih per j-chunk so gx_0's chunk j waits
        # only on its own slice; whh_0 isn't needed until layer 0 step 1
        nc.sync.dma_start(bias_bufs[0][:], biasT_d[0])
        if nl > 1:
            nc.sync.dma_start(bias_bufs[1][:], biasT_d[1])
        for jp in range(3):
            js = slice(jp * 2, (jp + 1) * 2)
            nc.sync.dma_start(
                wih_sb[:, js, :],
                wihT8_d[0].rearrange("p (k g) -> p k g", k=kt)[:, js, :])
        nc.sync.dma_start(
            whh_bufs[0][:], whhT8_d[0].rearrange("p (k g) -> p k g", k=kt))
        emit_bias_broadcast(0)
        if nl > 1:
            emit_bias_broadcast(1)

        emit_gx(0, list(range(mt)))

        # ---- layers ----
        TANH = AF.Tanh
        SIG = AF.Sigmoid
        groups = [(0, 1), (2, 3), (4, 5)]
        for l in range(nl):
            WHH = whh_bufs[l % 2]
            c_cur = None
            # fc1 accumulators (layer 9 only), held across the whole layer
            if l == nl - 1:
                ps_fc1 = [psG.tile([64, fn1], F32, tag="gps", name=f"fc1ps_{n}")
                          for n in range(2)]
            for t in range(w):
                m, p0 = t // 2, 64 * (t % 2)
                c_new = cp.tile([64, h], F32)
                hh = hp.tile([64, h], BF16)

                def chunk_tail(j, gt, c_prev):
                    # c/h update for h-slice j (gates chunk j); emitted right
                    # after chunk j's gate ACTs so the slice tails pipeline
                    # with later chunks' matmuls/ACTs
                    hs = slice(j * 128, (j + 1) * 128)
                    i_g, f_g = gt[:, 0:128], gt[:, 128:256]
                    g_g, o_g = gt[:, 256:384], gt[:, 384:512]
                    if c_prev is None:
                        nc.vector.tensor_tensor(c_new[:, hs], i_g, g_g, OP.mult)
                    else:
                        t1 = tp.tile([64, 128], F32, tag="t1")
                        nc.vector.tensor_tensor(t1[:], i_g, g_g, OP.mult)
                        t2 = tp.tile([64, 128], F32, tag="t2")
                        nc.gpsimd.tensor_tensor(t2[:], f_g, c_prev[:, hs], OP.mult)
                        nc.vector.tensor_tensor(c_new[:, hs], t1[:], t2[:], OP.add)
                    tc_t = tp.tile([64, 128], F32, tag="tc")
                    nc.scalar.activation(tc_t[:], c_new[:, hs], TANH)
                    nc.gpsimd.tensor_tensor(hh[:, hs], o_g, tc_t[:], OP.mult)
                    pending.append((t, j, hh))

                if t == 0:
                    for j in range(nch):
                        gt = gatep.tile([64, 512], F32, tag="gt")
                        gxt, ml = gx_ap(m)
                        src = gxt[p0:p0 + 64, ml, j * 512:(j + 1) * 512]
                        nc.scalar.activation(gt[:, 0:256], src[:, 0:256], SIG)
                        nc.scalar.activation(gt[:, 256:384], src[:, 256:384], TANH)
                        nc.scalar.activation(gt[:, 384:512], src[:, 384:512], SIG)
                        chunk_tail(j, gt, None)
                else:
                    hT8_prev = XT8[:, :, (t - 1) * 64:t * 64]
                    lagged = []  # (j, gt) tails emitted one group late so the
                                 # ACT stream isn't stalled on DVE results
                    for gi, grp in enumerate(groups):
                        pss = {j: psR.tile([128, 512], F32, tag="rps", name=f"recps_{l}_{t}_{j}")
                               for j in grp}
                        for j in grp:
                            js = slice(j * 512, (j + 1) * 512)
                            gxt, ml = gx_ap(m)
                            nc.tensor.matmul(
                                pss[j][0:64, :], idb32[p0:p0 + 64, :],
                                gxt[p0:p0 + 64, ml, js],
                                start=True, stop=False)
                        for i in range(kt // 2):
                            if gi == 0:
                                drain_upto(min(2 * i + 3, kt - 1))
                            for j in grp:
                                js = slice(j * 512, (j + 1) * 512)
                                nc.tensor.matmul(
                                    pss[j][0:64, :],
                                    hT8_prev[:, 2 * i:2 * i + 2, :],
                                    WHH[:, 2 * i:2 * i + 2, js],
                                    start=False, stop=(i == kt // 2 - 1),
                                    perf_mode=DR)
                        inv = 1.0 / WSCALE
                        for j in grp:
                            gt = gatep.tile([64, 512], F32, tag="gt")
                            ps = pss[j]
                            nc.scalar.activation(gt[:, 0:256], ps[0:64, 0:256], SIG, scale=inv)
                            nc.scalar.activation(gt[:, 256:384], ps[0:64, 256:384], TANH, scale=inv)
                            nc.scalar.activation(gt[:, 384:512], ps[0:64, 384:512], SIG, scale=inv)
                            lagged.append((j, gt))
                        if gi > 0:
                            for (jj, gg) in lagged[:2]:
                                chunk_tail(jj, gg, c_cur)
                            lagged = lagged[2:]
                    for (jj, gg) in lagged:
                        chunk_tail(jj, gg, c_cur)
                # this layer's last gx m-tile (m4, from the previous layer's
                # steps 8,9) — emitted after t=0's gate/tail block so the PE
                # work overlaps the t=0 ACT chain instead of preceding it
                if t == 0 and l > 0:
                    emit_gx(l, [4])
                # prefetch next layer's weights once per layer, early
                if t == 0 and l + 1 < nl:
                    nc.sync.dma_start(
                        whh_bufs[(l + 1) % 2][:],
                        whhT8_d[l + 1].rearrange("p (k g) -> p k g", k=kt))
                    for jp in range(3):
                        js = slice(jp * 2, (jp + 1) * 2)
                        nc.sync.dma_start(
                            wih_sb[:, js, :],
                            wihT8_d[l + 1].rearrange("p (k g) -> p k g", k=kt)[:, js, :])
                    if l + 2 < nl:
                        nc.sync.dma_start(bias_bufs[(l + 2) % 2][:], biasT_d[l + 2])
                        emit_bias_broadcast(l + 2)
                c_cur = c_new
                drain_upto(0)

                if l + 1 < nl:
                    if t % 2 == 1 and t < w - 1:
                        emit_gx(l + 1, [t // 2])
                else:
                    # fc1 partial: contract XT[:, s, t-slice] for this t
                    for s in range(kt):
                        drain_upto(s + 1)
                        for n in range(2):
                            ns = slice(n * fn1, (n + 1) * fn1)
                            wk = wsp.tile([128, fn1], BF16, tag="fc1w")
                            nc.sync.dma_start(
                                wk[:], fc1wT_d[(t * kt + s) * 128:(t * kt + s + 1) * 128, ns])
                            nc.tensor.matmul(
                                ps_fc1[n][:], XT[:, s, t * 64:(t + 1) * 64], wk[:],
                                start=(t == 0 and s == 0), stop=False)

        # ---- fc1 finalize ----
        out_sb = outp.tile([64, h], F32)
        for n in range(2):
            ns = slice(n * fn1, (n + 1) * fn1)
            nc.tensor.matmul(
                ps_fc1[n][:], onesb[:, 0:64], fc1b_sb[:, ns], start=False, stop=True)
            nc.vector.tensor_copy(out_sb[:, ns], ps_fc1[n][:])
        nc.sync.dma_start(out_d[:], out_sb[:])

    nc.compile()
    return nc


def _gate_perm(h=H):
    """Column permutation: new chunk j = [i_j | f_j | g_j | o_j] (128 each)."""
    g4 = 4 * h
    kt = h // 128
    perm = np.empty(g4, np.int64)
    for j in range(kt):
        for q in range(4):
            perm[j * 512 + q * 128:j * 512 + (q + 1) * 128] = np.arange(
                q * h + j * 128, q * h + (j + 1) * 128)
    return perm


def prep_inputs_one(inputs, h=H, w=W, nl=L, bl=BL, core=None, x_batch=None):
    """Host-side prep for one shard. inputs keyed as in setup_inputs()."""
    f32 = np.float32
    bf16 = ml_dtypes.bfloat16
    g4 = 4 * h
    if x_batch is None:
        x = np.ascontiguousarray(np.asarray(inputs["x_position"], f32)).reshape(-1, w, h)
        x_batch = x[core * bl:(core + 1) * bl]
    xposT = np.ascontiguousarray(x_batch.transpose(2, 1, 0).reshape(h, w * bl))
    perm = _gate_perm(h)
    f8 = ml_dtypes.float8_e4m3fn
    wihT = np.asarray(inputs["w_ih"], f32).transpose(0, 2, 1)[:, :, perm]
    whhT = np.asarray(inputs["w_hh"], f32).transpose(0, 2, 1)[:, :, perm]
    biasT = (np.asarray(inputs["b_ih"], f32) + np.asarray(inputs["b_hh"], f32))[:, perm]

    def dr_layout(wT):
        # [nl, h, g4] -> [nl, 128, 6*g4]: row p, block i, plane q holds
        # w[h = 256*i + 128*q + p, :], scaled by 32 for fp8 range
        out = (wT * 32.0).reshape(nl, 3, 2, 128, g4).transpose(0, 3, 1, 2, 4)
        return np.ascontiguousarray(out.reshape(nl, 128, 6 * g4)).astype(f8)

    return {
        "xposT": xposT.astype(bf16),
        "fc0wT": np.ascontiguousarray(np.asarray(inputs["fc0_w"], f32).T).astype(bf16),
        "fc0b": np.ascontiguousarray(
            np.asarray(inputs["fc0_b"], f32).reshape(1, h)).astype(bf16),
        "wihT8": dr_layout(wihT),
        "whhT8": dr_layout(whhT),
        "biasT": np.ascontiguousarray(biasT.reshape(nl, 1, g4)).astype(bf16),
        "fc1wT": np.ascontiguousarray(np.asarray(inputs["fc1_w"], f32).T).astype(bf16),
        "fc1bT": np.ascontiguousarray(
            np.asarray(inputs["fc1_b"], f32).reshape(1, h)).astype(bf16),
    }


def prep_inputs(inputs):
    shared = prep_inputs_one(inputs, core=0)
    x = np.ascontiguousarray(np.asarray(inputs["x_position"], np.float32)).reshape(-1, W, H)
    in_maps = [dict(shared)]
    for c in range(1, NCORES):
        m = dict(shared)
        xc = x[c * BL:(c + 1) * BL]
        m["xposT"] = np.ascontiguousarray(
            xc.transpose(2, 1, 0).reshape(H, W * BL)).astype(ml_dtypes.bfloat16)
        in_maps.append(m)
    return in_maps


def run_on_cores(in_maps, trace=False, **kwargs):
    from concourse.bass_utils import run_bass_kernel_spmd
    if "nc" not in _CACHE:
        _CACHE["nc"] = build_program()
    nc = _CACHE["nc"]
    return run_bass_kernel_spmd(
        nc, in_maps, core_ids=list(range(NCORES)), trace=trace, **kwargs)


def kernel(**inputs) -> np.ndarray:
    in_maps = prep_inputs(inputs)
    res = run_on_cores(in_maps)
    outs = [res.results[c]["out"] for c in range(NCORES)]
    full = np.concatenate(outs, axis=0)            # (512, 768)
    return np.ascontiguousarray(full.reshape(-1, 3).astype(np.float32))


# revision 35
# speedup vs baseline: 1.4330x; 1.1784x over previous
"""Trainium2 Bass kernel for nn_BasicLSTM: fc0 -> 10x LSTM(768) -> fc1.

Strategy: data-parallel over the 512 windows across 8 cores (64 windows each).
The two big GEMMs (w_hh recurrence, w_ih input side) run in fp8e4m3 with
DoubleRow perf mode (weights host-scaled by 32 to clear the fp8 subnormal
range, descaled for free via the gate ACTs' scale operand / the bias-fold
STT); everything else bf16 with f32 PSUM/elementwise.  Measured end-to-end
rel err ~1.04e-2 on HW vs the f32 reference (gate 2e-2).

Schedule (per core), TimelineSim ~1.04 ms with PE and ACT co-saturated:

  - Gate columns are host-permuted into an interleaved layout: 512-col chunk j
    holds [i_j | f_j | g_j | o_j] for h-slice j (128 cols each).  The c/h
    update for slice j starts right after chunk j's matmuls — the elementwise
    tail at each step covers one slice, not the whole gate row.
  - gx (input-side GEMM, batched over (t, b) step pairs with M=128
    stationaries) for layer l+1 is interleaved into layer l's recurrence as
    one burst per m-tile, right after step 2m+1 produced that step pair.  The
    bursts fill the PE idle left by the serial recurrence chain.  The m4
    burst is emitted after the NEXT layer's t=0 gate block (its inputs are
    the previous layer's steps 8,9, still intact in XT), so its PE work
    overlaps the t=0 ACT chain.  One gx buffer, overwritten in place after
    consumption (program order + tile dependency tracking make this safe).
    w_ih stays resident in a single SBUF buffer, reloaded once per layer
    right after the m4 burst that consumes the previous contents.
  - The "+ gx_t" term is injected into each rec PSUM chunk by a 64x64
    identity matmul (start=True) before the w_hh matmuls accumulate on top.
    This kills the per-step DVE adds and the odd-step partition-shift DMAs of
    the naive layout (odd steps live at partitions 64-127 of gx; a matmul rhs
    can read there, a DVE tensor_tensor against partitions 0-63 cannot).
    (A DVE PSUM-preload variant measured slower: it puts the copy on the
    critical path of every chunk's matmuls.)
  - Rec matmuls are emitted in groups of 2 chunks, k-major within a group.
    h-slice transposes are deferred ("pending") and drained with one-k-tile
    lookahead (slice k's transpose+copy issue a full matmul ahead of the
    consumer), plus a slice-0 pre-drain at each step end, so PE never waits
    on the c/h tail or on the DVE XT-writeback latency.
  - The gx bias add is folded into the PSUM->SBUF copy as a DVE tensor_tensor
    against a bias tile pre-broadcast to all 128 partitions (removes 30 bias
    matmuls per layer).
  - w_hh for layer l+1 and biases prefetch during layer l; fc1 is interleaved
    into layer 9's steps with its two PSUM accumulators held across the layer.
  - Gate quarters are ordered [i|f|o|g] per chunk so ONE sigmoid ACT covers
    384 cols (i,f,o) and one tanh covers g — 12 ACT instructions per step
    instead of 18.  Elementwise work is spread across ACT (gates, tanh c),
    DVE (i*g, c add, PSUM->SBUF copies incl. the fp8 XT8 twin — GpSimd has
    no PSUM port), and GpSimd (f*c, h mult, SBUF-only).
"""
import numpy as np
import ml_dtypes

H = 768
G = 4 * H          # 3072
W = 10             # time steps (window size)
L = 10             # layers
B_FULL = 512
NCORES = 8
BL = B_FULL // NCORES  # 64 windows per core

_CACHE = {}


def build_program(h=H, w=W, nl=L, bl=BL):
    import concourse.mybir as mybir
    import concourse.tile as tile
    from concourse import bacc
    from concourse.masks import make_identity

    F32 = mybir.dt.float32
    BF16 = mybir.dt.bfloat16
    FP8 = mybir.dt.float8e4
    DR = mybir.MatmulPerfMode.DoubleRow
    WSCALE = 32.0
    AF = mybir.ActivationFunctionType
    OP = mybir.AluOpType

    g4 = 4 * h
    kt = h // 128           # k-tiles over h (6)
    nch = g4 // 512         # 512-wide chunks over the gate dim (6)
    mt = (w * bl) // 128    # m-tiles over the (t, b) axis (5)
    fh = w * h              # fc1 contraction size
    fn1 = h // 2            # fc1 output chunk (two psum chunks)
    assert h % 128 == 0 and g4 % 512 == 0 and (w * bl) % 128 == 0 and bl == 64
    assert kt == nch  # chunk j's gates act on h-slice j (interleaved layout)

    nc = bacc.Bacc("TRN2", target_bir_lowering=False, debug=False)

    xposT_d = nc.dram_tensor("xposT", [h, w * bl], BF16, kind="ExternalInput")
    fc0wT_d = nc.dram_tensor("fc0wT", [h, h], BF16, kind="ExternalInput")
    fc0b_d = nc.dram_tensor("fc0b", [1, h], BF16, kind="ExternalInput")
    # fp8 weights in DoubleRow block layout: row p, block i (of 3), plane q
    # (of 2) holds w[h = 256*i + 128*q + p, gate], pre-scaled by 32 on the host
    # (sigma=0.02 weights sit in fp8e4m3's subnormal range unscaled)
    wihT8_d = nc.dram_tensor("wihT8", [nl, 128, 6 * g4], FP8, kind="ExternalInput")
    whhT8_d = nc.dram_tensor("whhT8", [nl, 128, 6 * g4], FP8, kind="ExternalInput")
    biasT_d = nc.dram_tensor("biasT", [nl, 1, g4], BF16, kind="ExternalInput")
    fc1wT_d = nc.dram_tensor("fc1wT", [fh, h], BF16, kind="ExternalInput")
    fc1bT_d = nc.dram_tensor("fc1bT", [1, h], BF16, kind="ExternalInput")
    out_d = nc.dram_tensor("out", [bl, h], F32, kind="ExternalOutput")

    with tile.TileContext(nc) as tc, \
         tc.tile_pool(name="persist", bufs=1) as pp, \
         tc.tile_pool(name="whhp", bufs=1) as whhp, \
         tc.tile_pool(name="biasp", bufs=1) as biasp, \
         tc.tile_pool(name="gxp", bufs=1) as gxp, \
         tc.tile_pool(name="wstream", bufs=6) as wsp, \
         tc.tile_pool(name="gatep", bufs=4) as gatep, \
         tc.tile_pool(name="tmp", bufs=3) as tp, \
         tc.tile_pool(name="cpool", bufs=2) as cp, \
         tc.tile_pool(name="hpool", bufs=2) as hp, \
         tc.tile_pool(name="outp", bufs=1) as outp, \
         tc.tile_pool(name="psR", bufs=4, space="PSUM") as psR, \
         tc.tile_pool(name="psG", bufs=2, space="PSUM") as psG, \
         tc.tile_pool(name="psT", bufs=2, space="PSUM") as psT:

        # ---- persistent tiles ----
        XT = pp.tile([128, kt, w * bl], BF16)      # h^T / layer-input storage
        onesb = pp.tile([1, 512], BF16)
        nc.vector.memset(onesb[:], 1.0)
        # identity in both partition halves: transposes + gx inject read it at
        # base partition 0 (even steps) or 64 (odd steps — matmul requires
        # lhsT and rhs to share a base partition)
        idb2 = pp.tile([128, 64], BF16)
        make_identity(nc, idb2[0:64, :])
        make_identity(nc, idb2[64:128, :])
        idb = idb2[0:64, :]
        fc0b_sb = pp.tile([1, h], BF16)
        nc.sync.dma_start(fc0b_sb[:], fc0b_d[:])
        fc1b_sb = pp.tile([1, h], BF16)
        nc.sync.dma_start(fc1b_sb[:], fc1bT_d[:])

        whh_bufs = [whhp.tile([128, kt, g4], FP8, name=f"whh{i}") for i in range(2)]
        bias_bufs = [biasp.tile([1, g4], BF16, name=f"bias{i}") for i in range(2)]
        # single-buffered resident w_ih: wih_{l+1} is loaded at layer l t=0 and
        # fully consumed by layer l's gx bursts, before layer l+1 t=0 reloads
        wih_sb = whhp.tile([128, kt, g4], FP8, name="wih_sb")
        # fp8 twin of XT (same slice indexing) — the DoubleRow lhsT for both
        # the rec and gx matmuls; plane pair for block i = slices 2i, 2i+1
        XT8 = pp.tile([128, kt, w * bl], FP8, name="XT8")
        # identity scaled by WSCALE for the gx inject, so the injected gx
        # matches the x32 weight scale (the gate ACTs then descale by 1/32);
        # the plain identity idb stays for transposes (CoreSim requires a
        # 0/1 permutation matrix there)
        idb32 = pp.tile([128, 64], BF16, name="idb32")
        make_identity(nc, idb32[0:64, :])
        make_identity(nc, idb32[64:128, :])
        nc.gpsimd.tensor_scalar_mul(idb32[:], idb32[:], WSCALE)
        # bias broadcast across all 128 partitions (both step parities), so the
        # gx PSUM->SBUF copy folds the bias add (no per-chunk bias matmuls)
        biasf_bufs = [biasp.tile([128, g4], BF16, name=f"biasf{i}") for i in range(2)]

        def emit_bias_broadcast(l):
            """biasf[l%2] <- bias_l broadcast to 128 partitions (via matmul)."""
            for j in range(nch):
                js = slice(j * 512, (j + 1) * 512)
                ps = psG.tile([128, 512], F32, tag="gps", name=f"biasbc_{l}_{j}")
                nc.tensor.matmul(
                    ps[:], onesb[:, 0:128], bias_bufs[l % 2][:, js],
                    start=True, stop=True)
                nc.vector.tensor_copy(biasf_bufs[l % 2][:, js], ps[:])

        # ---- fc0: XT <- fc0_wT.T @ xposT + fc0_b ----
        fc0w = gxp.tile([128, kt, h], BF16, tag="gxA", name="fc0w")
        xpt = gxp.tile([128, kt, w * bl], BF16, tag="gxB", name="xpt")
        for k in range(kt):
            nc.sync.dma_start(
                xpt[:, k, :], xposT_d.rearrange("(k p) c -> p k c", p=128)[:, k, :])
            nc.sync.dma_start(
                fc0w[:, k, :], fc0wT_d.rearrange("(k p) ho -> p k ho", p=128)[:, k, :])
        fc0_chunks = [(c, min(512, w * bl - c)) for c in range(0, w * bl, 512)]
        for m in range(kt):
            for c0, cw in fc0_chunks:
                ps = psG.tile([128, 512], F32, tag="gps", name=f"fc0ps_{m}_{c0}")
                for k in range(kt):
                    nc.tensor.matmul(
                        ps[:, :cw],
                        fc0w[:, k, m * 128:(m + 1) * 128],
                        xpt[:, k, c0:c0 + cw],
                        start=(k == 0), stop=False,
                    )
                nc.tensor.matmul(
                    ps[:, :cw], fc0b_sb[:, m * 128:(m + 1) * 128],
                    onesb[:, 0:cw], start=False, stop=True)
                nc.vector.tensor_copy(XT[:, m, c0:c0 + cw], ps[:, :cw])
                nc.scalar.copy(XT8[:, m, c0:c0 + cw], ps[:, :cw])

        # gx buffer, split in two tiles tag-sharing the SBUF slots of fc0's
        # staging tiles above (fc0's lifetime ends before gx is first written;
        # allocation order matches program order so slot versioning is clean).
        # even steps at partitions 0-63, odd at 64-127.
        gxA = gxp.tile([128, 3, g4], BF16, tag="gxA", name="gxA")
        gxB = gxp.tile([128, mt - 3, g4], BF16, tag="gxB", name="gxB")

        def gx_ap(m):
            return (gxA, m) if m < 3 else (gxB, m - 3)

        pending = []  # deferred (t, j, hh) transposes, drained into the next
                      # consumer's matmul stream just before the k-tile that
                      # needs slice j, so PE never stalls on the c/h tail

        def drain_upto(k):
            while pending and pending[0][1] <= k:
                tt, j, hh_t = pending.pop(0)
                trp = psT.tile([128, 64], BF16, tag="tps", name=f"trp_{tt}_{j}")
                nc.tensor.transpose(trp[:], hh_t[:, j * 128:(j + 1) * 128], idb[:])
                nc.vector.tensor_copy(XT[:, j, tt * 64:(tt + 1) * 64], trp[:])
                nc.scalar.copy(XT8[:, j, tt * 64:(tt + 1) * 64], trp[:])

        def emit_gx(l, mlist):
            """gx[m] <- XT[:, :, m-tile].T @ wih_sb + bias_l, for m in mlist.
            wih_sb must hold wihT_d[l]; biasf[l%2] must hold bias_l broadcast."""
            biasf = biasf_bufs[l % 2]
            drain_upto(0)
            for j in range(nch):
                js = slice(j * 512, (j + 1) * 512)
                pss = {m: psG.tile([128, 512], F32, tag="gps", name=f"gxps_{l}_{j}_{m}")
                       for m in mlist}
                for i in range(kt // 2):
                    if j == 0:
                        # lookahead drain: block i consumes slices 2i, 2i+1
                        drain_upto(min(2 * i + 3, kt - 1))
                    for m in mlist:
                        nc.tensor.matmul(
                            pss[m][:],
                            XT8[:, 2 * i:2 * i + 2, m * 128:(m + 1) * 128],
                            wih_sb[:, 2 * i:2 * i + 2, js],
                            start=(i == 0), stop=(i == kt // 2 - 1),
                            perf_mode=DR)
                for m in mlist:
                    gxt, ml = gx_ap(m)
                    nc.vector.scalar_tensor_tensor(
                        gxt[:, ml, js], pss[m][:], 1.0 / WSCALE, biasf[:, js],
                        OP.mult, OP.add)

        # prefetch layer 0 weights; wih per j-chunk so gx_0's chunk j waits
        # only on its own slice; whh_0 isn't needed until layer 0 step 1
        nc.sync.dma_start(bias_bufs[0][:], biasT_d[0])
        if nl > 1:
            nc.sync.dma_start(bias_bufs[1][:], biasT_d[1])
        for jp in range(3):
            js = slice(jp * 2, (jp + 1) * 2)
            nc.sync.dma_start(
                wih_sb[:, js, :],
                wihT8_d[0].rearrange("p (k g) -> p k g", k=kt)[:, js, :])
        nc.sync.dma_start(
            whh_bufs[0][:], whhT8_d[0].rearrange("p (k g) -> p k g", k=kt))
        emit_bias_broadcast(0)
        if nl > 1:
            emit_bias_broadcast(1)

        emit_gx(0, list(range(mt)))

        # ---- layers ----
        TANH = AF.Tanh
        SIG = AF.Sigmoid
        groups = [(0, 1), (2, 3), (4, 5)]
        for l in range(nl):
            WHH = whh_bufs[l % 2]
            c_cur = None
            # fc1 accumulators (layer 9 only), held across the whole layer
            if l == nl - 1:
                ps_fc1 = [psG.tile([64, fn1], F32, tag="gps", name=f"fc1ps_{n}")
                          for n in range(2)]
            for t in range(w):
                m, p0 = t // 2, 64 * (t % 2)
                c_new = cp.tile([64, h], F32)
                hh = hp.tile([64, h], BF16)

                def chunk_tail(j, gt, c_prev):
                    # c/h update for h-slice j (gates chunk j); emitted right
                    # after chunk j's gate ACTs so the slice tails pipeline
                    # with later chunks' matmuls/ACTs
                    hs = slice(j * 128, (j + 1) * 128)
                    i_g, f_g = gt[:, 0:128], gt[:, 128:256]
                    g_g, o_g = gt[:, 256:384], gt[:, 384:512]
                    if c_prev is None:
                        nc.vector.tensor_tensor(c_new[:, hs], i_g, g_g, OP.mult)
                    else:
                        t1 = tp.tile([64, 128], F32, tag="t1")
                        nc.vector.tensor_tensor(t1[:], i_g, g_g, OP.mult)
                        t2 = tp.tile([64, 128], F32, tag="t2")
                        nc.gpsimd.tensor_tensor(t2[:], f_g, c_prev[:, hs], OP.mult)
                        nc.vector.tensor_tensor(c_new[:, hs], t1[:], t2[:], OP.add)
                    tc_t = tp.tile([64, 128], F32, tag="tc")
                    nc.scalar.activation(tc_t[:], c_new[:, hs], TANH)
                    nc.gpsimd.tensor_tensor(hh[:, hs], o_g, tc_t[:], OP.mult)
                    pending.append((t, j, hh))

                if t == 0:
                    for j in range(nch):
                        gt = gatep.tile([64, 512], F32, tag="gt")
                        gxt, ml = gx_ap(m)
                        src = gxt[p0:p0 + 64, ml, j * 512:(j + 1) * 512]
                        nc.scalar.activation(gt[:, 0:256], src[:, 0:256], SIG)
                        nc.scalar.activation(gt[:, 256:384], src[:, 256:384], TANH)
                        nc.scalar.activation(gt[:, 384:512], src[:, 384:512], SIG)
                        chunk_tail(j, gt, None)
                else:
                    hT8_prev = XT8[:, :, (t - 1) * 64:t * 64]
                    lagged = []  # (j, gt) tails emitted one group late so the
                                 # ACT stream isn't stalled on DVE results
                    for gi, grp in enumerate(groups):
                        pss = {j: psR.tile([128, 512], F32, tag="rps", name=f"recps_{l}_{t}_{j}")
                               for j in grp}
                        for j in grp:
                            js = slice(j * 512, (j + 1) * 512)
                            gxt, ml = gx_ap(m)
                            nc.tensor.matmul(
                                pss[j][0:64, :], idb32[p0:p0 + 64, :],
                                gxt[p0:p0 + 64, ml, js],
                                start=True, stop=False)
                        for i in range(kt // 2):
                            if gi == 0:
                                drain_upto(min(2 * i + 3, kt - 1))
                            for j in grp:
                                js = slice(j * 512, (j + 1) * 512)
                                nc.tensor.matmul(
                                    pss[j][0:64, :],
                                    hT8_prev[:, 2 * i:2 * i + 2, :],
                                    WHH[:, 2 * i:2 * i + 2, js],
                                    start=False, stop=(i == kt // 2 - 1),
                                    perf_mode=DR)
                        inv = 1.0 / WSCALE
                        for j in grp:
                            gt = gatep.tile([64, 512], F32, tag="gt")
                            ps = pss[j]
                            nc.scalar.activation(gt[:, 0:256], ps[0:64, 0:256], SIG, scale=inv)
                            nc.scalar.activation(gt[:, 256:384], ps[0:64, 256:384], TANH, scale=inv)
                            nc.scalar.activation(gt[:, 384:512], ps[0:64, 384:512], SIG, scale=inv)
                            lagged.append((j, gt))
                        if gi > 0:
                            for (jj, gg) in lagged[:2]:
                                chunk_tail(jj, gg, c_cur)
                            lagged = lagged[2:]
                    for (jj, gg) in lagged:
                        chunk_tail(jj, gg, c_cur)
                # this layer's last gx m-tile (m4, from the previous layer's
                # steps 8,9) — emitted after t=0's gate/tail block so the PE
                # work overlaps the t=0 ACT chain instead of preceding it
                if t == 0 and l > 0:
                    emit_gx(l, [4])
                # prefetch next layer's weights once per layer, early
                if t == 0 and l + 1 < nl:
                    nc.sync.dma_start(
                        whh_bufs[(l + 1) % 2][:],
                        whhT8_d[l + 1].rearrange("p (k g) -> p k g", k=kt))
                    for jp in range(3):
                        js = slice(jp * 2, (jp + 1) * 2)
                        nc.sync.dma_start(
                            wih_sb[:, js, :],
                            wihT8_d[l + 1].rearrange("p (k g) -> p k g", k=kt)[:, js, :])
                    if l + 2 < nl:
                        nc.sync.dma_start(bias_bufs[(l + 2) % 2][:], biasT_d[l + 2])
                        emit_bias_broadcast(l + 2)
                c_cur = c_new
                drain_upto(0)

                if l + 1 < nl:
                    if t % 2 == 1 and t < w - 1:
                        emit_gx(l + 1, [t // 2])
                else:
                    # fc1 partial: contract XT[:, s, t-slice] for this t
                    for s in range(kt):
                        drain_upto(s + 1)
                        for n in range(2):
                            ns = slice(n * fn1, (n + 1) * fn1)
                            wk = wsp.tile([128, fn1], BF16, tag="fc1w")
                            nc.sync.dma_start(
                                wk[:], fc1wT_d[(t * kt + s) * 128:(t * kt + s + 1) * 128, ns])
                            nc.tensor.matmul(
                                ps_fc1[n][:], XT[:, s, t * 64:(t + 1) * 64], wk[:],
                                start=(t == 0 and s == 0), stop=False)

        # ---- fc1 finalize ----
        out_sb = outp.tile([64, h], F32)
        for n in range(2):
            ns = slice(n * fn1, (n + 1) * fn1)
            nc.tensor.matmul(
                ps_fc1[n][:], onesb[:, 0:64], fc1b_sb[:, ns], start=False, stop=True)
            nc.vector.tensor_copy(out_sb[:, ns], ps_fc1[n][:])
        nc.sync.dma_start(out_d[:], out_sb[:])

    nc.compile()
    return nc


def _gate_perm(h=H):
    """Column permutation: new chunk j = [i_j | f_j | g_j | o_j] (128 each)."""
    g4 = 4 * h
    kt = h // 128
    perm = np.empty(g4, np.int64)
    for j in range(kt):
        for q in range(4):
            perm[j * 512 + q * 128:j * 512 + (q + 1) * 128] = np.arange(
                q * h + j * 128, q * h + (j + 1) * 128)
    return perm


def prep_inputs_one(inputs, h=H, w=W, nl=L, bl=BL, core=None, x_batch=None):
    """Host-side prep for one shard. inputs keyed as in setup_inputs()."""
    f32 = np.float32
    bf16 = ml_dtypes.bfloat16
    g4 = 4 * h
    if x_batch is None:
        x = np.ascontiguousarray(np.asarray(inputs["x_position"], f32)).reshape(-1, w, h)
        x_batch = x[core * bl:(core + 1) * bl]
    xposT = np.ascontiguousarray(x_batch.transpose(2, 1, 0).reshape(h, w * bl))
    perm = _gate_perm(h)
    f8 = ml_dtypes.float8_e4m3fn
    wihT = np.asarray(inputs["w_ih"], f32).transpose(0, 2, 1)[:, :, perm]
    whhT = np.asarray(inputs["w_hh"], f32).transpose(0, 2, 1)[:, :, perm]
    biasT = (np.asarray(inputs["b_ih"], f32) + np.asarray(inputs["b_hh"], f32))[:, perm]

    def dr_layout(wT):
        # [nl, h, g4] -> [nl, 128, 6*g4]: row p, block i, plane q holds
        # w[h = 256*i + 128*q + p, :], scaled by 32 for fp8 range
        out = (wT * 32.0).reshape(nl, 3, 2, 128, g4).transpose(0, 3, 1, 2, 4)
        return np.ascontiguousarray(out.reshape(nl, 128, 6 * g4)).astype(f8)

    return {
        "xposT": xposT.astype(bf16),
        "fc0wT": np.ascontiguousarray(np.asarray(inputs["fc0_w"], f32).T).astype(bf16),
        "fc0b": np.ascontiguousarray(
            np.asarray(inputs["fc0_b"], f32).reshape(1, h)).astype(bf16),
        "wihT8": dr_layout(wihT),
        "whhT8": dr_layout(whhT),
        "biasT": np.ascontiguousarray(biasT.reshape(nl, 1, g4)).astype(bf16),
        "fc1wT": np.ascontiguousarray(np.asarray(inputs["fc1_w"], f32).T).astype(bf16),
        "fc1bT": np.ascontiguousarray(
            np.asarray(inputs["fc1_b"], f32).reshape(1, h)).astype(bf16),
    }


def prep_inputs(inputs):
    shared = prep_inputs_one(inputs, core=0)
    x = np.ascontiguousarray(np.asarray(inputs["x_position"], np.float32)).reshape(-1, W, H)
    in_maps = [dict(shared)]
    for c in range(1, NCORES):
        m = dict(shared)
        xc = x[c * BL:(c + 1) * BL]
        m["xposT"] = np.ascontiguousarray(
            xc.transpose(2, 1, 0).reshape(H, W * BL)).astype(ml_dtypes.bfloat16)
        in_maps.append(m)
    return in_maps


def run_on_cores(in_maps, trace=False, **kwargs):
    from concourse.bass_utils import run_bass_kernel_spmd
    if "nc" not in _CACHE:
        _CACHE["nc"] = build_program()
    nc = _CACHE["nc"]
    return run_bass_kernel_spmd(
        nc, in_maps, core_ids=list(range(NCORES)), trace=trace, **kwargs)


def kernel(**inputs) -> np.ndarray:
    in_maps = prep_inputs(inputs)
    res = run_on_cores(in_maps)
    outs = [res.results[c]["out"] for c in range(NCORES)]
    full = np.concatenate(outs, axis=0)            # (512, 768)
    return np.ascontiguousarray(full.reshape(-1, 3).astype(np.float32))
